# revision 1
# baseline (speedup 1.0000x reference)
"""ESM2 contact predictor head on 8 Trainium2 NeuronCores.

Computes out[b, i, j] = sigmoid(x[b,i] @ W @ x[b,j] + bias) for
x: (8, 2050, 320) f32, W: (320, 320) f32, bias: (1,) f32.

Sharding: data-parallel over batch — core c handles batch element c.

Per-core algorithm (all matmuls in float32r: full PE stream rate with
near-fp32 accuracy; PSUM accumulates in fp32):
  host:  xt = x[c].T as 3 K-slabs of 128 partitions (D=320 zero-padded
         to 384), pre-chunked so input DMAs are contiguous;
         wp = W zero-padded to (384, 384).
  chip:  warmup matmuls release the PE clock-gate while inputs stream in;
         u = wp.T @ xt                      == (x[c] @ W).T
         out[i, j] = sigmoid(sum_e u[e,i] * xt[e,j] + bias), produced as
         128-row x 1024-col half-strips: 6 matmuls (k-outer, shared
         stationary operand, alternating PSUM banks) -> one fused
         sigmoid+bias on ScalarE reading PSUM -> DMA out.
         The 2 tail columns (j=2048:2050) are computed transposed
         (2 partitions x 2050) into a separate DRAM tensor; the host
         transposes them into place.
         Early strips' first halves interleave with the tail of phase 1
         so the PE never waits on input DMA.
"""

import numpy as np

import concourse.mybir as mybir
import concourse.tile as tile
from concourse import bacc
from concourse.bass_utils import run_bass_kernel_spmd

N_CORES = 8
B, L, D = 8, 2050, 320
KT = 3            # K slabs: 128, 128, 64(duplicated)
F32 = mybir.dt.float32
F32R = mybir.dt.float32r
SIG = mybir.ActivationFunctionType.Sigmoid

J_TAIL = 2048
CHUNK = 512       # input DMA chunk (columns)

_cache = {}


def _build(bias_val: float):
    nc = bacc.Bacc("TRN2", target_bir_lowering=False, debug=False,
                   num_devices=N_CORES)
    xt_main_d = nc.dram_tensor("xt_main", [4, 128, KT, CHUNK], F32R,
                               kind="ExternalInput")
    xt_tail_d = nc.dram_tensor("xt_tail", [128, KT, 2], F32R,
                               kind="ExternalInput")
    w_d = nc.dram_tensor("w", [384, 384], F32R, kind="ExternalInput")
    out_d = nc.dram_tensor("out", [L, J_TAIL], F32, kind="ExternalOutput")
    outt_d = nc.dram_tensor("out_tail_t", [2, L], F32, kind="ExternalOutput")

    w_r = w_d.ap().rearrange("(k p) e -> p k e", p=128)     # (128, 3, 384)

    with tile.TileContext(nc) as tc:
        with (
            tc.tile_pool(name="persist", bufs=1) as pp,
            tc.tile_pool(name="outp", bufs=8) as outp,
            tc.tile_pool(name="psum", bufs=2, space="PSUM") as psp,
        ):
            bias_t = pp.tile([128, 1], F32)
            nc.vector.memset(bias_t[:], bias_val)

            w_sb = pp.tile([128, KT, 384], F32R)
            xt_sb = pp.tile([128, KT, L], F32R)
            nc.sync.dma_start(w_sb[:, 0], w_r[:, 0])
            nc.sync.dma_start(xt_sb[:, :, 0:CHUNK], xt_main_d.ap()[0])
            nc.sync.dma_start(w_sb[:, 1], w_r[:, 1])
            nc.sync.dma_start(w_sb[:, 2], w_r[:, 2])
            nc.sync.dma_start(xt_sb[:, :, CHUNK:2 * CHUNK], xt_main_d.ap()[1])
            nc.sync.dma_start(xt_sb[:, :, 2 * CHUNK:3 * CHUNK], xt_main_d.ap()[2])
            nc.sync.dma_start(xt_sb[:, :, 3 * CHUNK:4 * CHUNK], xt_main_d.ap()[3])
            nc.sync.dma_start(xt_sb[:, :, J_TAIL:L], xt_tail_d.ap())

            u_sb = pp.tile([128, KT, L], F32R)

            # PE warmup: dummy matmuls during the input-DMA window so the
            # HAM clock-gate is released before real work starts; one psum
            # tile, each matmul its own start/stop group.
            warm_sb = pp.tile([128, 512], F32R)
            nc.vector.memset(warm_sb.bitcast(F32)[:], 1.0)
            psw = psp.tile([128, 1024], F32, tag="pair", bufs=3, name="psw")
            for wi in range(12):
                h = (wi % 2) * 512
                nc.tensor.matmul(psw[:, h:h + 512], lhsT=warm_sb[:, :128],
                                 rhs=warm_sb[:], start=True, stop=True)
            # preload the sigmoid activation table while DMAs run
            act_warm = pp.tile([128, 1], F32)
            nc.scalar.activation(act_warm[:], bias_t[:], SIG)

            def mm_acc(ps, lhs, j0, nsz):
                """One accumulation group over the 3 K slabs (slab 2
                zero-padded to K=128)."""
                for k in range(KT):
                    nc.tensor.matmul(ps, lhsT=lhs(k),
                                     rhs=xt_sb[:, k, j0:j0 + nsz],
                                     start=(k == 0), stop=(k == KT - 1))

            # ---- phase 1: u = wp.T @ xt  (u[e, i], e on partitions) ----
            def wlhs_of(et):
                def wlhs(k, e0=et * 128):
                    return w_sb[:, k, e0:e0 + 128]
                return wlhs

            def ph1_ntp(ntp):
                n0 = ntp * 1024
                for et in range(KT):
                    ps1 = psp.tile([128, 1024], F32, tag="pair", bufs=3,
                                   name="ps1")
                    mm_acc(ps1[:, 0:512], wlhs_of(et), n0, 512)
                    mm_acc(ps1[:, 512:1024], wlhs_of(et), n0 + 512, 512)
                    nc.vector.tensor_copy(u_sb[:, et, n0:n0 + 1024], ps1[:, :])

            def ph1_tail():
                for et in range(KT):
                    psT = psp.tile([128, 512], F32, tag="small", bufs=2,
                                   name="psT")
                    mm_acc(psT[:, :2], wlhs_of(et), J_TAIL, 2)
                    nc.vector.tensor_copy(u_sb[:, et, J_TAIL:L], psT[:, :2])

            def tail_block():
                # out[:, 2048:2050] for all i, computed transposed (j on
                # partitions), written to its own DRAM tensor; host
                # transposes it into place.
                outT = outp.tile([2, L], F32, tag="ttail", bufs=1, name="outT")
                for ic in range(4):
                    c0 = ic * 512
                    psc = psp.tile([128, 512], F32, tag="small", bufs=2,
                                   name="psc")
                    for k in range(KT):
                        nc.tensor.matmul(psc[:2, :512],
                                         lhsT=xt_sb[:, k, J_TAIL:L],
                                         rhs=u_sb[:, k, c0:c0 + 512],
                                         start=(k == 0), stop=(k == KT - 1))
                    nc.scalar.activation(outT[:, c0:c0 + 512], psc[:2, :512],
                                         SIG, bias=bias_t[:2, :])
                psc2 = psp.tile([128, 512], F32, tag="small", bufs=2,
                                name="psc2")
                for k in range(KT):
                    nc.tensor.matmul(psc2[:2, :2], lhsT=xt_sb[:, k, J_TAIL:L],
                                     rhs=u_sb[:, k, J_TAIL:L],
                                     start=(k == 0), stop=(k == KT - 1))
                nc.scalar.activation(outT[:, J_TAIL:L], psc2[:2, :2], SIG,
                                     bias=bias_t[:2, :])
                nc.sync.dma_start(outt_d.ap()[:], outT[:])

            def do_half(i0, isz, jp, fine_dma=False):
                # one 1024-column half of an output strip: 6 matmuls
                # (k-outer, shared lhsT, alternating banks) -> sigmoid -> DMA
                a0 = jp * 1024
                half = outp.tile([128, 1024], F32, tag="strip", bufs=8,
                                 name="half")
                ps = psp.tile([128, 1024], F32, tag="pair", bufs=3, name="ps")
                for k in range(KT):
                    u_k = u_sb[:, k, i0:i0 + isz]
                    nc.tensor.matmul(ps[:isz, 0:512], lhsT=u_k,
                                     rhs=xt_sb[:, k, a0:a0 + 512],
                                     start=(k == 0), stop=(k == KT - 1))
                    nc.tensor.matmul(ps[:isz, 512:1024], lhsT=u_k,
                                     rhs=xt_sb[:, k, a0 + 512:a0 + 1024],
                                     start=(k == 0), stop=(k == KT - 1))
                nc.scalar.activation(half[:isz, :], ps[:isz, :], SIG,
                                     bias=bias_t[:isz, :])
                if fine_dma:
                    for q in range(2):
                        nc.sync.dma_start(
                            out_d.ap()[i0:i0 + isz,
                                       a0 + q * 512:a0 + (q + 1) * 512],
                            half[:isz, q * 512:(q + 1) * 512])
                else:
                    nc.sync.dma_start(out_d.ap()[i0:i0 + isz, a0:a0 + 1024],
                                      half[:isz, :])

            # Interleaved schedule: the first strips' jp=0 halves only need
            # input chunks 0-1 and u columns 0:1024, so they run while
            # chunks 2-3 are still arriving.
            ph1_ntp(0)
            for s in range(8):
                do_half(s * 128, 128, 0)
            ph1_ntp(1)
            ph1_tail()
            tail_block()
            for s in range(8):
                do_half(s * 128, 128, 1)
            for s in range(8, 15):
                do_half(s * 128, 128, 0)
                do_half(s * 128, 128, 1)
            do_half(15 * 128, 128, 0, fine_dma=True)
            do_half(15 * 128, 128, 1, fine_dma=True)
            do_half(2048, 2, 0)
            do_half(2048, 2, 1)

    nc.compile()
    return nc


last_results = None


def _host_pack(x, W):
    xT = x.transpose(0, 2, 1)  # (B, 320, 2050)
    full = np.empty((B, 128, KT, L), np.float32)
    full[:, :, 0, :] = xT[:, 0:128]
    full[:, :, 1, :] = xT[:, 128:256]
    full[:, 0:64, 2, :] = xT[:, 256:320]
    full[:, 64:128, 2, :] = 0.0
    xt_main = np.ascontiguousarray(
        full[..., :J_TAIL].reshape(B, 128, KT, 4, CHUNK)
        .transpose(0, 3, 1, 2, 4))
    xt_tail = np.ascontiguousarray(full[..., J_TAIL:L])
    wp = np.zeros((384, 384), np.float32)
    wp[0:320, 0:320] = W
    return xt_main, xt_tail, wp


def kernel(x, W, b, _trace=False):
    global last_results
    x = np.ascontiguousarray(np.asarray(x, dtype=np.float32))
    W = np.asarray(W, dtype=np.float32)
    b = np.asarray(b, dtype=np.float32)
    bias_val = float(b[0])

    if bias_val not in _cache:
        _cache.clear()
        _cache[bias_val] = _build(bias_val)
    nc = _cache[bias_val]

    xt_main, xt_tail, wp = _host_pack(x, W)
    in_maps = [{"xt_main": xt_main[c], "xt_tail": xt_tail[c], "w": wp}
               for c in range(N_CORES)]
    res = run_bass_kernel_spmd(nc, in_maps, core_ids=list(range(N_CORES)),
                               trace=_trace)
    last_results = res
    out = np.empty((B, L, L), dtype=np.float32)
    for c in range(N_CORES):
        out[c, :, :J_TAIL] = res.results[c]["out"]
        out[c, :, J_TAIL:] = res.results[c]["out_tail_t"].T
    return out



# revision 4
# speedup vs baseline: 1.0474x; 1.0474x over previous
"""ESM2 contact predictor head on 8 Trainium2 NeuronCores.

Computes out[b, i, j] = sigmoid(x[b,i] @ W @ x[b,j] + bias) for
x: (8, 2050, 320) f32, W: (320, 320) f32, bias: (1,) f32.

Sharding: data-parallel over batch — core c handles batch element c.

Per-core algorithm (fp16 operands, f32 PSUM accumulation — rel err
~7e-4 vs the f32 reference; fp16 streams at the same 1 col/cycle PE
rate as fp32r but halves LDWEIGHTS and input-DMA time):
  host:  xt[p, k, j] = x[j, 128k+p] as 5 contiguous chunks of 410
         columns (D=320 zero-padded to 384 = 3 K-slabs of 128);
         w = W zero-padded to (384, 384), slab-major.
  chip:  warmup matmuls release the PE clock-gate while inputs stream;
         ph1: u^T[e, i] = sum_d W[d,e] xt[d,i]  (3 e-blocks x 3 K-slabs
              per 410-col chunk, PSUM f32 -> fp16 u_sb via DVE cast)
         ph2: 16 row-strips (M=128) + one M=2 tail strip, each 5
              j-blocks of 410 cols; consecutive matmuls always target
              alternating PSUM banks (same-bank back-to-back matmuls
              stall ~105 ns on the accumulate turnaround), so j-blocks
              are processed in pairs k-outer, and the odd 5th j-block
              is paired across adjacent strips. Fused sigmoid+bias on
              ScalarE (one op per j-block pair) -> contiguous row DMA
              into the (2050, 2050) output.
         ph1 chunks are interleaved between early strips so the PE
         never idles while the input DMAs finish.
"""

import numpy as np

import concourse.mybir as mybir
import concourse.tile as tile
from concourse import bacc
from concourse.bass_utils import run_bass_kernel_spmd

N_CORES = 8
B, L, D = 8, 2050, 320
KT = 3                  # K slabs (zero-padded to 3 x 128)
NCH, CW = 5, 410        # j/i chunking: 5 x 410 = 2050 exactly
F32 = mybir.dt.float32
F16 = mybir.dt.float16
SIG = mybir.ActivationFunctionType.Sigmoid
NWARM = 8

_cache = {}


def _build(bias_val: float):
    nc = bacc.Bacc("TRN2", target_bir_lowering=False, debug=False,
                   num_devices=N_CORES)
    xt_d = nc.dram_tensor("xt", [NCH, 128, KT, CW], F16, kind="ExternalInput")
    w_d = nc.dram_tensor("w", [128, KT, 384], F16, kind="ExternalInput")
    out_d = nc.dram_tensor("out", [L, L], F32, kind="ExternalOutput")

    with tile.TileContext(nc) as tc:
        with (
            tc.tile_pool(name="persist", bufs=1) as pp,
            tc.tile_pool(name="outp", bufs=4) as outp,
            tc.tile_pool(name="psum", bufs=4, space="PSUM") as psp,
        ):
            bias_t = pp.tile([128, 1], F32)
            nc.vector.memset(bias_t[:], bias_val)

            w_sb = pp.tile([128, KT, 384], F16)
            xt_sb = pp.tile([128, KT, NCH, CW], F16)
            u_sb = pp.tile([128, KT, L], F16)

            nc.sync.dma_start(w_sb[:], w_d.ap())
            for c in range(NCH):
                nc.sync.dma_start(xt_sb[:, :, c, :], xt_d.ap()[c])

            # PE warmup: dummy matmuls release the HAM clock-gate and burn
            # the p-state ramp while the input DMAs land.
            warm_sb = pp.tile([128, 512], F16)
            nc.vector.memset(warm_sb[:], 0.0)
            psw = psp.tile([128, 2, 512], F32, tag="ps", name="psw")
            for wi in range(NWARM):
                nc.tensor.matmul(psw[:, wi % 2, :], lhsT=warm_sb[:, :128],
                                 rhs=warm_sb[:], start=True, stop=True)
            # preload the sigmoid activation table while DMAs run
            act_warm = pp.tile([128, 1], F32)
            nc.scalar.activation(act_warm[:], bias_t[:], SIG)

            def mm_pair(lhs_of, rhs_a, rhs_b, ps, m):
                # 6 matmuls, k-outer, alternating the two PSUM banks so no
                # two consecutive matmuls hit the same bank.
                for k in range(KT):
                    u_k = lhs_of(k)
                    nc.tensor.matmul(ps[:m, 0, 0:CW], lhsT=u_k,
                                     rhs=rhs_a(k), start=(k == 0),
                                     stop=(k == KT - 1))
                    nc.tensor.matmul(ps[:m, 1, 0:CW], lhsT=u_k,
                                     rhs=rhs_b(k), start=(k == 0),
                                     stop=(k == KT - 1))

            def ph1(c):
                # u^T[e, 410c:410c+410] for all e, from xt chunk c
                def wl(et):
                    return lambda k: w_sb[:, k, 128 * et:128 * et + 128]

                def xr(k, _c=c):
                    return xt_sb[:, k, _c, :]

                ps = psp.tile([128, 2, 512], F32, tag="ps", name="ph1p")
                for k in range(KT):
                    nc.tensor.matmul(ps[:, 0, 0:CW], lhsT=wl(0)(k), rhs=xr(k),
                                     start=(k == 0), stop=(k == KT - 1))
                    nc.tensor.matmul(ps[:, 1, 0:CW], lhsT=wl(1)(k), rhs=xr(k),
                                     start=(k == 0), stop=(k == KT - 1))
                nc.vector.tensor_copy(u_sb[:, 0, CW * c:CW * (c + 1)],
                                      ps[:, 0, 0:CW])
                nc.vector.tensor_copy(u_sb[:, 1, CW * c:CW * (c + 1)],
                                      ps[:, 1, 0:CW])

            def ph1_et2(c):
                # et=2 is a single output block, so its accumulation cannot
                # alternate banks: accept the small same-bank stall.
                ps = psp.tile([128, 2, 512], F32, tag="ps", name="ph1s")
                for k in range(KT):
                    nc.tensor.matmul(ps[:, 0, 0:CW],
                                     lhsT=w_sb[:, k, 256:384],
                                     rhs=xt_sb[:, k, c, :],
                                     start=(k == 0), stop=(k == KT - 1))
                nc.vector.tensor_copy(u_sb[:, 2, CW * c:CW * (c + 1)],
                                      ps[:, 0, 0:CW])

            strip_out = {}

            def strip_main(i0, m):
                # j-blocks 0..3 of one output row-strip (rows i0:i0+m)
                outt = outp.tile([128, NCH, CW], F32, tag="strip", bufs=4,
                                 name="outt")
                strip_out[i0] = outt

                def ul(k):
                    return u_sb[:, k, i0:i0 + m]

                for half in range(2):
                    ps = psp.tile([128, 2, 512], F32, tag="ps", name="ps")
                    mm_pair(ul,
                            lambda k, c=2 * half: xt_sb[:, k, c, :],
                            lambda k, c=2 * half + 1: xt_sb[:, k, c, :],
                            ps, m)
                    nc.scalar.activation(outt[:m, 2 * half:2 * half + 2, :],
                                         ps[:m, :, 0:CW], SIG,
                                         bias=bias_t[:m, :])
                    nc.sync.dma_start(
                        out_d.ap()[i0:i0 + m, 820 * half:820 * half + 820],
                        outt[:m, 2 * half:2 * half + 2, :])

            def strip_j4(i0a, m_a, i0b, m_b):
                # the odd 5th j-block (cols 1640:2050) for two row-strips,
                # paired so consecutive matmuls alternate PSUM banks.
                ps = psp.tile([128, 2, 512], F32, tag="ps", name="ps4")
                for k in range(KT):
                    nc.tensor.matmul(ps[:m_a, 0, 0:CW],
                                     lhsT=u_sb[:, k, i0a:i0a + m_a],
                                     rhs=xt_sb[:, k, 4, :],
                                     start=(k == 0), stop=(k == KT - 1))
                    nc.tensor.matmul(ps[:m_b, 1, 0:CW],
                                     lhsT=u_sb[:, k, i0b:i0b + m_b],
                                     rhs=xt_sb[:, k, 4, :],
                                     start=(k == 0), stop=(k == KT - 1))
                for sub, (i0, m) in enumerate(((i0a, m_a), (i0b, m_b))):
                    outt = strip_out.pop(i0)
                    nc.scalar.activation(outt[:m, 4, :], ps[:m, sub, 0:CW],
                                         SIG, bias=bias_t[:m, :])
                    nc.sync.dma_start(out_d.ap()[i0:i0 + m, 1640:2050],
                                      outt[:m, 4, :])

            def strip_j4_solo(i0, m):
                ps = psp.tile([128, 2, 512], F32, tag="ps", name="ps4")
                for k in range(KT):
                    nc.tensor.matmul(ps[:m, 0, 0:CW],
                                     lhsT=u_sb[:, k, i0:i0 + m],
                                     rhs=xt_sb[:, k, 4, :],
                                     start=(k == 0), stop=(k == KT - 1))
                outt = strip_out.pop(i0)
                nc.scalar.activation(outt[:m, 4, :], ps[:m, 0, 0:CW], SIG,
                                     bias=bias_t[:m, :])
                nc.sync.dma_start(out_d.ap()[i0:i0 + m, 1640:2050],
                                  outt[:m, 4, :])

            def full_ph1(c):
                ph1(c)
                ph1_et2(c)

            # Interleave ph1 chunks between early strips: strip s only
            # needs u columns i0:i0+128, so ph1 stays a chunk ahead while
            # later input chunks are still arriving.
            full_ph1(0)
            full_ph1(1)
            strip_main(0, 128)
            full_ph1(2)
            strip_main(128, 128)
            strip_j4(0, 128, 128, 128)
            full_ph1(3)
            strip_main(256, 128)
            full_ph1(4)
            strip_main(384, 128)
            strip_j4(256, 128, 384, 128)
            for sp in range(2, 8):
                a, b = 2 * sp, 2 * sp + 1
                strip_main(128 * a, 128)
                strip_main(128 * b, 128)
                strip_j4(128 * a, 128, 128 * b, 128)
            strip_main(2048, 2)
            strip_j4_solo(2048, 2)

    nc.compile()
    return nc


last_results = None


def _host_pack(x, W):
    xT = x.transpose(0, 2, 1)  # (B, 320, L)
    full = np.zeros((B, 128, KT, L), np.float16)
    full[:, :, 0, :] = xT[:, 0:128]
    full[:, :, 1, :] = xT[:, 128:256]
    full[:, 0:64, 2, :] = xT[:, 256:320]
    xt_in = np.ascontiguousarray(
        full.reshape(B, 128, KT, NCH, CW).transpose(0, 3, 1, 2, 4))
    Wp = np.zeros((384, 384), np.float16)
    Wp[0:320, 0:320] = W.astype(np.float16)
    w_in = np.ascontiguousarray(Wp.reshape(KT, 128, 384).transpose(1, 0, 2))
    return xt_in, w_in


def kernel(x, W, b, _trace=False):
    global last_results
    x = np.ascontiguousarray(np.asarray(x, dtype=np.float32))
    W = np.asarray(W, dtype=np.float32)
    b = np.asarray(b, dtype=np.float32)
    bias_val = float(b[0])

    if bias_val not in _cache:
        _cache.clear()
        _cache[bias_val] = _build(bias_val)
    nc = _cache[bias_val]

    xt_in, w_in = _host_pack(x, W)
    in_maps = [{"xt": xt_in[c], "w": w_in} for c in range(N_CORES)]
    res = run_bass_kernel_spmd(nc, in_maps, core_ids=list(range(N_CORES)),
                               trace=_trace)
    last_results = res
    out = np.empty((B, L, L), dtype=np.float32)
    for c in range(N_CORES):
        out[c] = res.results[c]["out"]
    return out
